# revision 40
# baseline (speedup 1.0000x reference)
"""Trainium2 Bass kernel for nn_MultiHeadAttention_64647847739885.

Reference semantics (fp32):
    Wq_eff = softmax(Wq + tril_mask, axis=-2)   (if maskout else Wq)  [H,D,DK]
    Wk_eff = softmax(Wk + tril_mask, axis=-2)
    WqQ = einsum('btd,hdk->bhtk', Q, Wq_eff)
    WkK = einsum('bsd,hdk->bhsk', K, Wk_eff)
    WvV = einsum('bsd,hdv->bhsv', V, Wv)
    scores = einsum('bhtk,bhsk->bhts', WqQ, WkK) / sqrt(dk)
    probs = softmax(scores, axis=-2)            # over the QUERY axis t!
    ctx = einsum('bhts,bhsv->bhtv', probs, WvV) -> (B,T,H*DV) @ Wo

Device strategy (8 NeuronCores, SPMD): core c owns batch b = c//2 and
head-group g = c%2 (8 heads each).  Each core computes attention + the
partial output projection (its 8 heads, all T rows) and the pair combines
partials with two pairwise bf16 ReduceScatters, each core emitting its
T/2 rows of the output.

Pipeline layout:
  - host casts: V/Wv/Wo travel bf16; Q/K and the pre-softmax Wq/Wk travel
    fp8e4 (safe: their rounding flows only through the tiny-score exp
    channel; V-side stays bf16 to protect the direct channel);
  - q/k projections contract via fp8 DoubleRow (two 128-deep k-subtiles
    per matmul); attention scores/ctx and everything else stay bf16;
  - DMA queues: SP carries wq/vT/qT/kT + output, Pool carries
    wk/wv/wo/consts; the ACT engine runs ONLY the exp chain;
  - attention emits both heads' scores matmuls adjacently on disjoint
    PE row-groups and both ctx matmuls on disjoint col-groups
    (tile_position), so the halves overlap in the array; ctx is deferred
    one s-tile and the next pair's projections interleave as PE fillers
    so the PE never waits on the exp;
  - psum: 2-deep scores ring (2x2 banks) + 1 filler group + ctx
    accumulator = 8 banks; mask uses -16 (exp(-16)~0, fp8-safe).
All softmax denominators fold into per-partition scales as in the
reference factorization.  Host does layout + dtype-cast work only.
"""

import numpy as np
import ml_dtypes

import concourse.bacc as bacc
import concourse.mybir as mybir
import concourse.tile as tile
from concourse import bass_utils
from concourse.bass_interp import get_hw_module

B, T, D = 4, 1024, 1024
H, DK = 16, 64
P = 128
N_CORES = 8
HC = 8               # heads per core
NPAIR = HC // 2      # head-pairs per core (ctx partition groups)
WCOLS = HC * DK      # packed weight columns per core (512)
ND = D // P          # contraction tiles for projections
NS = T // P          # s tiles
NT2 = T // 512       # moving-dim halves

F32 = mybir.dt.float32
BF16 = mybir.dt.bfloat16
FP8 = mybir.dt.float8e4
BFNP = ml_dtypes.bfloat16
F8NP = ml_dtypes.float8_e4m3

RG_PAIRS = [[0, 1], [2, 3], [4, 5], [6, 7]]

EXP = mybir.ActivationFunctionType.Exp


def _emit_body(nc, tc, aps, pools, maskout, use_rs, rep):
    qT, kT, vT, wq, wk, wv, wo, tri, ones, ones8d, out = aps
    pp, tp, op_, psb, psf, psc = pools

    ones_t = pp.tile([P, 1], BF16, tag="ones")
    ones8 = pp.tile([P, 1], FP8, tag="ones8")
    qT_t = pp.tile([P, ND, T], FP8, tag="qT")
    kT_t = pp.tile([P, ND, T], FP8, tag="kT")
    vT_t = pp.tile([P, ND, T], BF16, tag="vT")
    wq_t = pp.tile([P, ND, WCOLS], FP8, tag="wq")
    wk_t = pp.tile([P, ND, WCOLS], FP8, tag="wk")
    wv_t = pp.tile([P, ND, WCOLS], BF16, tag="wv")
    wvv = pp.tile([P, NS, WCOLS], BF16, tag="wvv")
    qq = pp.tile([P, NPAIR, T], BF16, tag="qq")
    kk = pp.tile([P, NPAIR, T], BF16, tag="kk")
    ctx = pp.tile([P, NPAIR, T], BF16, tag="ctx")
    wo_t = pp.tile([P, NPAIR, D], BF16, tag="wo")
    wst_q = pp.tile([P, ND, WCOLS], FP8, tag="wstq")
    wst_k = pp.tile([P, ND, WCOLS], FP8, tag="wstk")
    if maskout:
        tri_t = pp.tile([P, WCOLS], FP8, tag="tri")

    # ---------------- DMA enqueues (SP / Pool queues) ---------------
    # SP: wq chunks -> vT -> qT -> kT (+ output later); Pool: consts,
    # wk, wv, wo.  ACT issues no DMA at all.
    wq_dst = wst_q
    wk_dst = wst_k
    if maskout:
        nc.gpsimd.dma_start(tri_t[:], tri[:])
    nc.gpsimd.dma_start(ones_t[:], ones[:])
    nc.gpsimd.dma_start(ones8[:], ones8d[:])
    for i in range(ND):
        nc.sync.dma_start(wq_dst[:, i, :], wq[i * P:(i + 1) * P, :])
    for i in range(ND):
        nc.sync.dma_start(vT_t[:, i, :], vT[i * P:(i + 1) * P, :])
    for i in range(ND):
        nc.gpsimd.dma_start(wk_dst[:, i, :], wk[i * P:(i + 1) * P, :])
    for i in range(ND):
        nc.gpsimd.dma_start(wv_t[:, i, :], wv[i * P:(i + 1) * P, :])
    for i in range(ND):
        nc.sync.dma_start(qT_t[:, i, :], qT[i * P:(i + 1) * P, :])
    for i in range(ND):
        nc.sync.dma_start(kT_t[:, i, :], kT[i * P:(i + 1) * P, :])
    for m in range(NPAIR):
        nc.gpsimd.dma_start(wo_t[:, m, :], wo[m * P:(m + 1) * P, :])

    # ---------------- weight softmax -------------------------------
    # additive mask (tri holds 0 / -1e4) then exp (ACT); the softmax
    # denominators become per-partition scales on qq via ones-matmul
    # column sums + PE transposes.
    cscale = [None] * NPAIR
    if maskout:
        nc.vector.tensor_add(wst_q[:, 0, :], wst_q[:, 0, :], tri_t[:])
        nc.vector.tensor_add(wst_k[:, 0, :], wst_k[:, 0, :], tri_t[:])
        for i in range(ND):
            nc.scalar.activation(wq_t[:, i, :], wst_q[:, i, :], EXP)
        for i in range(ND):
            nc.scalar.activation(wk_t[:, i, :], wst_k[:, i, :], EXP)
    else:
        for i in range(ND):
            nc.vector.tensor_copy(wq_t[:, i, :], wst_q[:, i, :])
        for i in range(ND):
            nc.vector.tensor_copy(wk_t[:, i, :], wst_k[:, i, :])

    # ---------------- softmax denominators -> cscale ----------------
    if maskout:
        ps_s = psf.tile([P, 1024], F32, tag="f")
        for i in range(ND):
            nc.tensor.matmul(ps_s[:1, 0:WCOLS], lhsT=ones8[:],
                             rhs=wq_t[:, i, :],
                             start=(i == 0), stop=(i == ND - 1))
        for i in range(ND):
            nc.tensor.matmul(ps_s[:1, WCOLS:T], lhsT=ones8[:],
                             rhs=wk_t[:, i, :],
                             start=(i == 0), stop=(i == ND - 1))
        ssb = tp.tile([1, T], BF16, tag="ssb")
        nc.vector.tensor_copy(ssb[:], ps_s[:1, :])
        ps_t = psf.tile([P, 1024], F32, tag="f")
        for pr in range(NPAIR):
            nc.tensor.matmul(ps_t[:, pr:pr + 1],
                             lhsT=ssb[:, pr * P:(pr + 1) * P],
                             rhs=ones_t[:1, :], start=True, stop=True)
            nc.tensor.matmul(
                ps_t[:, 4 + pr:5 + pr],
                lhsT=ssb[:, WCOLS + pr * P:WCOLS + (pr + 1) * P],
                rhs=ones_t[:1, :], start=True, stop=True)
        sqk = tp.tile([P, 2 * NPAIR], F32, tag="sqk")
        nc.vector.tensor_copy(sqk[:], ps_t[:, 0:2 * NPAIR])
        prod = tp.tile([P, NPAIR], F32, tag="prod")
        nc.vector.tensor_mul(prod[:], sqk[:, 0:NPAIR], sqk[:, NPAIR:])
        call = pp.tile([P, NPAIR], F32, tag="call")
        nc.vector.reciprocal(call[:], prod[:])
        for pr in range(NPAIR):
            cscale[pr] = call[:, pr:pr + 1]

    # ---------------- wvv = (V @ Wv) in (s x v), bf16 ---------------
    for st in range(NS):
        ps = psb.tile([P, 1024], F32, tag="big")
        for i in range(ND):
            nc.tensor.matmul(ps[:, :WCOLS],
                             lhsT=vT_t[:, i, st * P:(st + 1) * P],
                             rhs=wv_t[:, i, :],
                             start=(i == 0), stop=(i == ND - 1))
        nc.vector.tensor_copy(wvv[:, st, :], ps[:, :WCOLS])

    # ---------------- q/k projection emitters -----------------------
    # Returned as a flat list of thunks (16 matmuls + evac) so the
    # attention loop can interleave them as PE fillers.
    def proj_thunks(pr, which, pool=None):
        w_t = wq_t if which == 'q' else wk_t
        x_t = qT_t if which == 'q' else kT_t
        dst = qq if which == 'q' else kk
        pool_, tag = (pool or psf), ("big" if pool is psb else "f")
        state = {}

        def mk(k):
            def f():
                if k == 0:
                    state['ps'] = pool_.tile([P, 1024], F32, tag=tag,
                                             name="ps_fill")
                j, n = divmod(k, NT2)
                # fp8 DoubleRow: two 128-deep k-subtiles per matmul
                nc.tensor.matmul(
                    state['ps'][:, n * 512:(n + 1) * 512],
                    lhsT=w_t[:, 2 * j:2 * j + 2, pr * P:(pr + 1) * P],
                    rhs=x_t[:, 2 * j:2 * j + 2, n * 512:(n + 1) * 512],
                    start=(j == 0), stop=(j == ND // 2 - 1),
                    perf_mode=mybir.MatmulPerfMode.DoubleRow)
            return f

        def evac():
            if which == 'q' and cscale[pr] is not None:
                nc.vector.tensor_scalar_mul(dst[:, pr, :], state['ps'][:],
                                            cscale[pr][:])
            else:
                nc.vector.tensor_copy(dst[:, pr, :], state['ps'][:])

        return [mk(k) for k in range(ND // 2 * NT2)] + [evac]

    def run_all(thunks):
        for t_ in thunks:
            t_()

    # ---------------- attention (software-pipelined) ----------------
    def attn_pair(pr, fillers):
        fi = 0
        pctx = psc.tile([P, T], F32, tag="ctxp")
        pend = None

        def emit_ctx2(e0, wvs0, e1, wvs1, st):
            # both halves adjacent: disjoint array col-groups overlap on HW
            for n in range(NT2):
                nc.tensor.matmul(
                    pctx[0:64, n * 512:(n + 1) * 512],
                    lhsT=wvs0[:], rhs=e0[:, n * 512:(n + 1) * 512],
                    start=(st == 0), stop=(st == NS - 1),
                    tile_position=(0, 0))
                nc.tensor.matmul(
                    pctx[64:128, n * 512:(n + 1) * 512],
                    lhsT=wvs1[:], rhs=e1[:, n * 512:(n + 1) * 512],
                    start=(st == 0), stop=(st == NS - 1),
                    tile_position=(0, 64))

        for st in range(NS):
            # scores for both heads adjacent: disjoint array row-groups
            ps0 = psb.tile([P, 1024], F32, tag="big", name="ps0")
            ps1 = psb.tile([P, 1024], F32, tag="big", name="ps1")
            for n in range(NT2):
                nc.tensor.matmul(
                    ps0[:, n * 512:(n + 1) * 512],
                    lhsT=kk[0:64, pr, st * P:(st + 1) * P],
                    rhs=qq[0:64, pr, n * 512:(n + 1) * 512],
                    start=True, stop=True, tile_position=(0, 0))
                nc.tensor.matmul(
                    ps1[:, n * 512:(n + 1) * 512],
                    lhsT=kk[64:128, pr, st * P:(st + 1) * P],
                    rhs=qq[64:128, pr, n * 512:(n + 1) * 512],
                    start=True, stop=True, tile_position=(64, 0))
            e0 = tp.tile([P, T], BF16, tag="e", name="e0")
            rs0 = tp.tile([P, 1], F32, tag="rs", name="rs0")
            nc.scalar.activation(e0[:], ps0[:], EXP, scale=0.125)
            nc.vector.tensor_scalar(e0[:], e0[:], 1.0, 0.0,
                                    mybir.AluOpType.mult,
                                    mybir.AluOpType.add,
                                    accum_out=rs0[:])
            r0 = tp.tile([P, 1], F32, tag="r", name="r0")
            nc.vector.reciprocal(r0[:], rs0[:])
            wvs0 = tp.tile([P, DK], BF16, tag="wvs", name="wvs0")
            hcol = 2 * pr * DK
            nc.vector.tensor_scalar_mul(wvs0[:],
                                        wvv[:, st, hcol:hcol + DK], r0[:])
            e1 = tp.tile([P, T], BF16, tag="e", name="e1")
            rs1 = tp.tile([P, 1], F32, tag="rs", name="rs1")
            nc.scalar.activation(e1[:], ps1[:], EXP, scale=0.125)
            nc.vector.tensor_scalar(e1[:], e1[:], 1.0, 0.0,
                                    mybir.AluOpType.mult,
                                    mybir.AluOpType.add,
                                    accum_out=rs1[:])
            r1 = tp.tile([P, 1], F32, tag="r", name="r1")
            nc.vector.reciprocal(r1[:], rs1[:])
            wvs1 = tp.tile([P, DK], BF16, tag="wvs", name="wvs1")
            hcol1 = (2 * pr + 1) * DK
            nc.vector.tensor_scalar_mul(wvs1[:],
                                        wvv[:, st, hcol1:hcol1 + DK], r1[:])
            nfill = 5 if fi < 30 else 4
            for _ in range(nfill):
                if fi < len(fillers):
                    fillers[fi]()
                    fi += 1
            if pend is not None:
                emit_ctx2(*pend)
            pend = (e0, wvs0, e1, wvs1, st)
        emit_ctx2(*pend)
        while fi < len(fillers):
            fillers[fi]()
            fi += 1
        nc.vector.tensor_copy(ctx[:, pr, :], pctx[:])

    run_all(proj_thunks(0, 'q'))
    run_all(proj_thunks(0, 'k', pool=psb))
    attn_pair(0, proj_thunks(1, 'q') + proj_thunks(1, 'k'))
    attn_pair(1, proj_thunks(2, 'q') + proj_thunks(2, 'k'))
    attn_pair(2, proj_thunks(3, 'q') + proj_thunks(3, 'k'))
    attn_pair(3, [])

    # ---------------- output projection (bf16 partial, all T rows) --
    # pairwise bf16 ReduceScatter combines head-group partials; rank r
    # of each pair receives rows [r*256,(r+1)*256) of each T/2 half.
    if use_rs:
        dp_cm = tc.tile_pool(name=f"dram{rep}", bufs=1, space="DRAM")
        dp = dp_cm.__enter__()
        obounce = dp.tile([T, D], BF16, tag="ob")
        ors1 = dp.tile([T // 4, D], BF16, tag="ors1")
        ors2 = dp.tile([T // 4, D], BF16, tag="ors2")
    for tt in range(T // P):
        pso = psb.tile([P, 1024], F32, tag="big")
        for m in range(NPAIR):
            for n in range(NT2):
                nc.tensor.matmul(
                    pso[:, n * 512:(n + 1) * 512],
                    lhsT=ctx[:, m, tt * P:(tt + 1) * P],
                    rhs=wo_t[:, m, n * 512:(n + 1) * 512],
                    start=(m == 0), stop=(m == NPAIR - 1))
        osb = op_.tile([P, D], BF16, tag="o")
        nc.vector.tensor_copy(osb[:], pso[:])
        if use_rs:
            nc.sync.dma_start(obounce[tt * P:(tt + 1) * P, :], osb[:])
            if tt == T // P // 2 - 1:
                # first-half RS overlaps the second half's projection
                nc.gpsimd.collective_compute(
                    "ReduceScatter", mybir.AluOpType.add,
                    replica_groups=RG_PAIRS,
                    ins=[obounce[0:T // 2, :].opt()], outs=[ors1.opt()])
                for q2 in range(2):
                    rb = op_.tile([P, D], BF16, tag="rb")
                    nc.sync.dma_start(rb[:], ors1[q2 * P:(q2 + 1) * P, :])
                    nc.sync.dma_start(out[q2 * P:(q2 + 1) * P, :], rb[:])
        else:
            nc.sync.dma_start(out[tt * P:(tt + 1) * P, :], osb[:])
    if use_rs:
        nc.gpsimd.collective_compute(
            "ReduceScatter", mybir.AluOpType.add,
            replica_groups=RG_PAIRS,
            ins=[obounce[T // 2:T, :].opt()], outs=[ors2.opt()])
        for q2 in range(2):
            rb = op_.tile([P, D], BF16, tag="rb")
            nc.sync.dma_start(rb[:], ors2[q2 * P:(q2 + 1) * P, :])
            nc.sync.dma_start(
                out[T // 4 + q2 * P:T // 4 + (q2 + 1) * P, :], rb[:])
        dp_cm.__exit__(None, None, None)


def _build(maskout: bool, heads_per_core: int = 8, use_rs: bool = True,
           repeat: int = 1, loop_reps: int = 0, phases=None):
    """Build + compile the SPMD program (signature kept for test.py)."""
    del heads_per_core, phases
    nc = bacc.Bacc("TRN2", target_bir_lowering=False, debug=False,
                   num_devices=N_CORES)

    OUT_ROWS = T // 2 if use_rs else T
    qT = nc.dram_tensor("qT", [D, T], FP8, kind="ExternalInput").ap()
    kT = nc.dram_tensor("kT", [D, T], FP8, kind="ExternalInput").ap()
    vT = nc.dram_tensor("vT", [D, T], BF16, kind="ExternalInput").ap()
    wq = nc.dram_tensor("wq", [D, WCOLS], FP8, kind="ExternalInput").ap()
    wk = nc.dram_tensor("wk", [D, WCOLS], FP8, kind="ExternalInput").ap()
    wv = nc.dram_tensor("wv", [D, WCOLS], BF16, kind="ExternalInput").ap()
    wo = nc.dram_tensor("wo", [WCOLS, D], BF16, kind="ExternalInput").ap()
    tri = nc.dram_tensor("tri", [P, WCOLS], FP8, kind="ExternalInput").ap()
    ones = nc.dram_tensor("ones", [P, 1], BF16, kind="ExternalInput").ap()
    ones8d = nc.dram_tensor("ones8d", [P, 1], FP8,
                            kind="ExternalInput").ap()
    out = nc.dram_tensor("out", [OUT_ROWS, D], BF16,
                         kind="ExternalOutput").ap()
    aps = (qT, kT, vT, wq, wk, wv, wo, tri, ones, ones8d, out)

    with tile.TileContext(nc) as tc:
        with (
            tc.tile_pool(name="persist", bufs=1) as pp,
            tc.tile_pool(name="trans", bufs=6) as tp,
            tc.tile_pool(name="osb", bufs=3) as op_,
            tc.tile_pool(name="psum_big", bufs=2, space="PSUM") as psb,
            tc.tile_pool(name="psum_fill", bufs=1, space="PSUM") as psf,
            tc.tile_pool(name="psum_ctx", bufs=1, space="PSUM") as psc,
        ):
            pools = (pp, tp, op_, psb, psf, psc)
            if loop_reps:
                assert not use_rs, "collectives cannot live inside For_i"
                with tc.For_i(0, loop_reps, 1):
                    _emit_body(nc, tc, aps, pools, maskout, use_rs, 0)
            else:
                for rep in range(repeat):
                    _emit_body(nc, tc, aps, pools, maskout, use_rs, rep)

    nc.compile()
    nc.m = get_hw_module(nc.m)
    return nc


_CACHE: dict = {}


def _get_program(maskout: bool):
    key = maskout
    if key not in _CACHE:
        _CACHE[key] = _build(maskout)
    return _CACHE[key]


def _prep_inputs(Q, K, V, Wq, Wk, Wv, Wo, heads_per_core=8):
    """Host-side sharding: layout + bf16 casts, per-core input dicts."""
    del heads_per_core
    keep = np.arange(P)[:, None] >= (np.arange(WCOLS)[None, :] % DK)
    tri = np.where(keep, 0.0, -16.0).astype(F8NP)
    ones = np.ones((P, 1), BFNP)
    ones8 = np.ones((P, 1), F8NP)
    in_maps = []
    for c in range(N_CORES):
        b, g = c // 2, c % 2
        hsel = np.arange(g * HC, (g + 1) * HC)
        wq_p = np.ascontiguousarray(
            Wq[hsel].transpose(1, 0, 2).reshape(D, WCOLS)).astype(F8NP)
        wk_p = np.ascontiguousarray(
            Wk[hsel].transpose(1, 0, 2).reshape(D, WCOLS)).astype(F8NP)
        wv_p = np.ascontiguousarray(
            Wv[hsel].transpose(1, 0, 2).reshape(D, WCOLS)).astype(BFNP)
        wo_p = np.ascontiguousarray(
            Wo.reshape(H, DK, D)[hsel].reshape(WCOLS, D)).astype(BFNP)
        in_maps.append({
            "qT": np.ascontiguousarray(Q[b].T).astype(F8NP),
            "kT": np.ascontiguousarray(K[b].T).astype(F8NP),
            "vT": np.ascontiguousarray(V[b].T).astype(BFNP),
            "wq": wq_p, "wk": wk_p, "wv": wv_p, "wo": wo_p,
            "tri": tri, "ones": ones, "ones8d": ones8,
        })
    return in_maps


def run(Q, K, V, Wq, Wk, Wv, Wo, maskout):
    Q = np.asarray(Q, np.float32)
    K = np.asarray(K, np.float32)
    V = np.asarray(V, np.float32)
    Wq = np.asarray(Wq, np.float32)
    Wk = np.asarray(Wk, np.float32)
    Wv = np.asarray(Wv, np.float32)
    Wo = np.asarray(Wo, np.float32)
    mk = bool(np.asarray(maskout).item())
    nc = _get_program(mk)
    in_maps = _prep_inputs(Q, K, V, Wq, Wk, Wv, Wo)
    res = bass_utils.run_bass_kernel_spmd(
        nc, in_maps, list(range(N_CORES)), trace=False)
    outf = np.empty((B, T, D), np.float32)
    for c in range(N_CORES):
        b, r = c // 2, c % 2
        o = np.asarray(res.results[c]["out"]).astype(np.float32)
        outf[b, r * (T // 4):(r + 1) * (T // 4), :] = o[:T // 4]
        outf[b, T // 2 + r * (T // 4):T // 2 + (r + 1) * (T // 4), :] = \
            o[T // 4:]
    return outf, res


def kernel(Q, K, V, Wq, Wk, Wv, Wo, maskout):
    outf, _ = run(Q, K, V, Wq, Wk, Wv, Wo, maskout)
    return outf


# revision 42
# speedup vs baseline: 1.1365x; 1.1365x over previous
"""Trainium2 Bass kernel for nn_MultiHeadAttention_64647847739885.

Reference semantics (fp32):
    Wq_eff = softmax(Wq + tril_mask, axis=-2)   (if maskout else Wq)  [H,D,DK]
    Wk_eff = softmax(Wk + tril_mask, axis=-2)
    WqQ = einsum('btd,hdk->bhtk', Q, Wq_eff)
    WkK = einsum('bsd,hdk->bhsk', K, Wk_eff)
    WvV = einsum('bsd,hdv->bhsv', V, Wv)
    scores = einsum('bhtk,bhsk->bhts', WqQ, WkK) / sqrt(dk)
    probs = softmax(scores, axis=-2)            # over the QUERY axis t!
    ctx = einsum('bhts,bhsv->bhtv', probs, WvV) -> (B,T,H*DV) @ Wo

Device strategy (8 NeuronCores, SPMD): core c owns batch b = c//2 and
head-group g = c%2 (8 heads each).  Each core computes attention + the
partial output projection (its 8 heads, all T rows) and the pair combines
partials with two pairwise bf16 ReduceScatters, each core emitting its
T/2 rows of the output.

Pipeline layout:
  - host casts: V/Wv/Wo travel bf16; Q/K and the pre-softmax Wq/Wk travel
    fp8e4 (safe: their rounding flows only through the tiny-score exp
    channel; V-side stays bf16 to protect the direct channel);
  - q/k projections contract via fp8 DoubleRow (two 128-deep k-subtiles
    per matmul); attention scores/ctx and everything else stay bf16;
  - DMA queues: SP carries wq/vT/qT/kT + output, Pool carries
    wk/wv/wo/consts; the ACT engine runs ONLY the exp chain;
  - attention emits both heads' scores matmuls adjacently on disjoint
    PE row-groups and both ctx matmuls on disjoint col-groups
    (tile_position), so the halves overlap in the array; ctx is deferred
    one s-tile and the next pair's projections interleave as PE fillers
    so the PE never waits on the exp;
  - psum: 2-deep scores ring (2x2 banks) + 1 filler group + ctx
    accumulator = 8 banks; mask uses -16 (exp(-16)~0, fp8-safe).
All softmax denominators fold into per-partition scales as in the
reference factorization.  Host does layout + dtype-cast work only.
"""

import numpy as np
import ml_dtypes

import concourse.bacc as bacc
import concourse.mybir as mybir
import concourse.tile as tile
from concourse import bass_utils
from concourse.bass_interp import get_hw_module

B, T, D = 4, 1024, 1024
H, DK = 16, 64
P = 128
N_CORES = 8
HC = 8               # heads per core
NPAIR = HC // 2      # head-pairs per core (ctx partition groups)
WCOLS = HC * DK      # packed weight columns per core (512)
ND = D // P          # contraction tiles for projections
NS = T // P          # s tiles
NT2 = T // 512       # moving-dim halves

F32 = mybir.dt.float32
BF16 = mybir.dt.bfloat16
FP8 = mybir.dt.float8e4
BFNP = ml_dtypes.bfloat16
F8NP = ml_dtypes.float8_e4m3

RG_PAIRS = [[0, 1], [2, 3], [4, 5], [6, 7]]

EXP = mybir.ActivationFunctionType.Exp


def _emit_body(nc, tc, aps, pools, maskout, use_rs, rep):
    qT, kT, vT, wq, wk, wv, wo, tri, ones, ones8d, out = aps
    pp, tp, op_, psb, psf, psc = pools

    ones_t = pp.tile([P, 1], BF16, tag="ones")
    ones8 = pp.tile([P, 1], FP8, tag="ones8")
    qT_t = pp.tile([P, ND, T], FP8, tag="qT")
    kT_t = pp.tile([P, ND, T], FP8, tag="kT")
    vT_t = pp.tile([P, ND, T], BF16, tag="vT")
    wq_t = pp.tile([P, ND, WCOLS], FP8, tag="wq")
    wk_t = pp.tile([P, ND, WCOLS], FP8, tag="wk")
    wv_t = pp.tile([P, ND, WCOLS], BF16, tag="wv")
    wvv = pp.tile([P, NS, WCOLS], BF16, tag="wvv")
    qq = pp.tile([P, NPAIR, T], BF16, tag="qq")
    kk = pp.tile([P, NPAIR, T], BF16, tag="kk")
    ctx = pp.tile([P, NPAIR, T], BF16, tag="ctx")
    wo_t = pp.tile([P, NPAIR, D], BF16, tag="wo")
    wst_q = pp.tile([P, ND, WCOLS], FP8, tag="wstq")
    wst_k = pp.tile([P, ND, WCOLS], FP8, tag="wstk")
    if maskout:
        tri_t = pp.tile([P, WCOLS], FP8, tag="tri")

    # ---------------- DMA enqueues (SP / Pool queues) ---------------
    # SP: wq chunks -> vT -> qT -> kT (+ output later); Pool: consts,
    # wk, wv, wo.  ACT issues no DMA at all.
    wq_dst = wst_q
    wk_dst = wst_k
    if maskout:
        nc.gpsimd.dma_start(tri_t[:], tri[:])
    nc.gpsimd.dma_start(ones_t[:], ones[:])
    nc.gpsimd.dma_start(ones8[:], ones8d[:])
    for i in range(ND):
        nc.sync.dma_start(wq_dst[:, i, :], wq[i * P:(i + 1) * P, :])
    for i in range(ND):
        nc.sync.dma_start(vT_t[:, i, :], vT[i * P:(i + 1) * P, :])
    for i in range(ND):
        nc.gpsimd.dma_start(wk_dst[:, i, :], wk[i * P:(i + 1) * P, :])
    for i in range(ND):
        nc.gpsimd.dma_start(wv_t[:, i, :], wv[i * P:(i + 1) * P, :])
    for i in range(ND):
        nc.sync.dma_start(qT_t[:, i, :], qT[i * P:(i + 1) * P, :])
    for i in range(ND):
        nc.sync.dma_start(kT_t[:, i, :], kT[i * P:(i + 1) * P, :])
    for m in range(NPAIR):
        nc.gpsimd.dma_start(wo_t[:, m, :], wo[m * P:(m + 1) * P, :])

    # ---------------- weight softmax -------------------------------
    # additive mask (tri holds 0 / -1e4) then exp (ACT); the softmax
    # denominators become per-partition scales on qq via ones-matmul
    # column sums + PE transposes.
    cscale = [None] * NPAIR
    if maskout:
        nc.vector.tensor_add(wst_q[:, 0, :], wst_q[:, 0, :], tri_t[:])
        nc.vector.tensor_add(wst_k[:, 0, :], wst_k[:, 0, :], tri_t[:])
        for i in range(ND):
            nc.scalar.activation(wq_t[:, i, :], wst_q[:, i, :], EXP)
        for i in range(ND):
            nc.scalar.activation(wk_t[:, i, :], wst_k[:, i, :], EXP)
    else:
        for i in range(ND):
            nc.vector.tensor_copy(wq_t[:, i, :], wst_q[:, i, :])
        for i in range(ND):
            nc.vector.tensor_copy(wk_t[:, i, :], wst_k[:, i, :])

    # ---------------- softmax denominators -> cscale ----------------
    if maskout:
        ps_s = psf.tile([P, 1024], F32, tag="f")
        for i in range(ND):
            nc.tensor.matmul(ps_s[:1, 0:WCOLS], lhsT=ones8[:],
                             rhs=wq_t[:, i, :],
                             start=(i == 0), stop=(i == ND - 1))
        for i in range(ND):
            nc.tensor.matmul(ps_s[:1, WCOLS:T], lhsT=ones8[:],
                             rhs=wk_t[:, i, :],
                             start=(i == 0), stop=(i == ND - 1))
        ssb = tp.tile([1, T], BF16, tag="ssb")
        nc.vector.tensor_copy(ssb[:], ps_s[:1, :])
        ps_t = psf.tile([P, 1024], F32, tag="f")
        for pr in range(NPAIR):
            nc.tensor.matmul(ps_t[:, pr:pr + 1],
                             lhsT=ssb[:, pr * P:(pr + 1) * P],
                             rhs=ones_t[:1, :], start=True, stop=True)
            nc.tensor.matmul(
                ps_t[:, 4 + pr:5 + pr],
                lhsT=ssb[:, WCOLS + pr * P:WCOLS + (pr + 1) * P],
                rhs=ones_t[:1, :], start=True, stop=True)
        sqk = tp.tile([P, 2 * NPAIR], F32, tag="sqk")
        nc.vector.tensor_copy(sqk[:], ps_t[:, 0:2 * NPAIR])
        prod = tp.tile([P, NPAIR], F32, tag="prod")
        nc.vector.tensor_mul(prod[:], sqk[:, 0:NPAIR], sqk[:, NPAIR:])
        call = pp.tile([P, NPAIR], F32, tag="call")
        nc.vector.reciprocal(call[:], prod[:])
        for pr in range(NPAIR):
            cscale[pr] = call[:, pr:pr + 1]

    # ---------------- wvv = (V @ Wv) in (s x v), bf16 ---------------
    for st in range(NS):
        ps = psb.tile([P, 1024], F32, tag="big")
        for i in range(ND):
            nc.tensor.matmul(ps[:, :WCOLS],
                             lhsT=vT_t[:, i, st * P:(st + 1) * P],
                             rhs=wv_t[:, i, :],
                             start=(i == 0), stop=(i == ND - 1))
        nc.vector.tensor_copy(wvv[:, st, :], ps[:, :WCOLS])

    # ---------------- q/k projection emitters -----------------------
    # Returned as a flat list of thunks (16 matmuls + evac) so the
    # attention loop can interleave them as PE fillers.
    def proj_thunks(pr, which, pool=None):
        w_t = wq_t if which == 'q' else wk_t
        x_t = qT_t if which == 'q' else kT_t
        dst = qq if which == 'q' else kk
        pool_, tag = (pool or psf), ("big" if pool is psb else "f")
        state = {}

        def mk(k):
            def f():
                if k == 0:
                    state['ps'] = pool_.tile([P, 1024], F32, tag=tag,
                                             name="ps_fill")
                j, n = divmod(k, NT2)
                # fp8 DoubleRow: two 128-deep k-subtiles per matmul
                nc.tensor.matmul(
                    state['ps'][:, n * 512:(n + 1) * 512],
                    lhsT=w_t[:, 2 * j:2 * j + 2, pr * P:(pr + 1) * P],
                    rhs=x_t[:, 2 * j:2 * j + 2, n * 512:(n + 1) * 512],
                    start=(j == 0), stop=(j == ND // 2 - 1),
                    perf_mode=mybir.MatmulPerfMode.DoubleRow)
            return f

        def evac():
            if which == 'q' and cscale[pr] is not None:
                nc.vector.tensor_scalar_mul(dst[:, pr, :], state['ps'][:],
                                            cscale[pr][:])
            else:
                nc.vector.tensor_copy(dst[:, pr, :], state['ps'][:])

        return [mk(k) for k in range(ND // 2 * NT2)] + [evac]

    def run_all(thunks):
        for t_ in thunks:
            t_()

    # ---------------- attention (software-pipelined) ----------------
    def attn_pair(pr, fillers):
        fi = 0
        pctx = psc.tile([P, T], F32, tag="ctxp")
        pend = None

        def emit_ctx2(e0, wvs0, e1, wvs1, st):
            # both halves adjacent: disjoint array col-groups overlap on HW
            for n in range(NT2):
                nc.tensor.matmul(
                    pctx[0:64, n * 512:(n + 1) * 512],
                    lhsT=wvs0[:], rhs=e0[:, n * 512:(n + 1) * 512],
                    start=(st == 0), stop=(st == NS - 1),
                    tile_position=(0, 0))
                nc.tensor.matmul(
                    pctx[64:128, n * 512:(n + 1) * 512],
                    lhsT=wvs1[:], rhs=e1[:, n * 512:(n + 1) * 512],
                    start=(st == 0), stop=(st == NS - 1),
                    tile_position=(0, 64))

        for st in range(NS):
            # scores for both heads adjacent: disjoint array row-groups
            ps0 = psb.tile([P, 1024], F32, tag="big", name="ps0")
            ps1 = psb.tile([P, 1024], F32, tag="big", name="ps1")
            # ps0's two matmuls first so exp0 starts one mm earlier;
            # ps1's row-group-64 matmuls still overlap ps0's in the array
            for n in range(NT2):
                nc.tensor.matmul(
                    ps0[:, n * 512:(n + 1) * 512],
                    lhsT=kk[0:64, pr, st * P:(st + 1) * P],
                    rhs=qq[0:64, pr, n * 512:(n + 1) * 512],
                    start=True, stop=True, tile_position=(0, 0))
            for n in range(NT2):
                nc.tensor.matmul(
                    ps1[:, n * 512:(n + 1) * 512],
                    lhsT=kk[64:128, pr, st * P:(st + 1) * P],
                    rhs=qq[64:128, pr, n * 512:(n + 1) * 512],
                    start=True, stop=True, tile_position=(64, 0))
            e0 = tp.tile([P, T], BF16, tag="e", name="e0")
            rs0 = tp.tile([P, 1], F32, tag="rs", name="rs0")
            nc.scalar.activation(e0[:], ps0[:], EXP, scale=0.125,
                                 accum_out=rs0[:])
            r0 = tp.tile([P, 1], F32, tag="r", name="r0")
            nc.vector.reciprocal(r0[:], rs0[:])
            wvs0 = tp.tile([P, DK], BF16, tag="wvs", name="wvs0")
            hcol = 2 * pr * DK
            nc.vector.tensor_scalar_mul(wvs0[:],
                                        wvv[:, st, hcol:hcol + DK], r0[:])
            e1 = tp.tile([P, T], BF16, tag="e", name="e1")
            rs1 = tp.tile([P, 1], F32, tag="rs", name="rs1")
            nc.scalar.activation(e1[:], ps1[:], EXP, scale=0.125,
                                 accum_out=rs1[:])
            r1 = tp.tile([P, 1], F32, tag="r", name="r1")
            nc.vector.reciprocal(r1[:], rs1[:])
            wvs1 = tp.tile([P, DK], BF16, tag="wvs", name="wvs1")
            hcol1 = (2 * pr + 1) * DK
            nc.vector.tensor_scalar_mul(wvs1[:],
                                        wvv[:, st, hcol1:hcol1 + DK], r1[:])
            nfill = 5 if fi < 30 else 4
            for _ in range(nfill):
                if fi < len(fillers):
                    fillers[fi]()
                    fi += 1
            if pend is not None:
                emit_ctx2(*pend)
            pend = (e0, wvs0, e1, wvs1, st)
        emit_ctx2(*pend)
        while fi < len(fillers):
            fillers[fi]()
            fi += 1
        nc.vector.tensor_copy(ctx[:, pr, :], pctx[:])

    run_all(proj_thunks(0, 'q'))
    run_all(proj_thunks(0, 'k', pool=psb))
    attn_pair(0, proj_thunks(1, 'q') + proj_thunks(1, 'k'))
    attn_pair(1, proj_thunks(2, 'q') + proj_thunks(2, 'k'))
    attn_pair(2, proj_thunks(3, 'q') + proj_thunks(3, 'k'))
    attn_pair(3, [])

    # ---------------- output projection (bf16 partial, all T rows) --
    # pairwise bf16 ReduceScatter combines head-group partials; rank r
    # of each pair receives rows [r*256,(r+1)*256) of each T/2 half.
    if use_rs:
        dp_cm = tc.tile_pool(name=f"dram{rep}", bufs=1, space="DRAM")
        dp = dp_cm.__enter__()
        obounce = dp.tile([T, D], BF16, tag="ob")
        ors1 = dp.tile([T // 4, D], BF16, tag="ors1")
        ors2 = dp.tile([T // 4, D], BF16, tag="ors2")
    for tt in range(T // P):
        pso = psb.tile([P, 1024], F32, tag="big")
        for m in range(NPAIR):
            for n in range(NT2):
                nc.tensor.matmul(
                    pso[:, n * 512:(n + 1) * 512],
                    lhsT=ctx[:, m, tt * P:(tt + 1) * P],
                    rhs=wo_t[:, m, n * 512:(n + 1) * 512],
                    start=(m == 0), stop=(m == NPAIR - 1))
        osb = op_.tile([P, D], BF16, tag="o")
        nc.vector.tensor_copy(osb[:], pso[:])
        if use_rs:
            nc.sync.dma_start(obounce[tt * P:(tt + 1) * P, :], osb[:])
            if tt == T // P // 2 - 1:
                # first-half RS overlaps the second half's projection
                nc.gpsimd.collective_compute(
                    "ReduceScatter", mybir.AluOpType.add,
                    replica_groups=RG_PAIRS,
                    ins=[obounce[0:T // 2, :].opt()], outs=[ors1.opt()])
                for q2 in range(2):
                    rb = op_.tile([P, D], BF16, tag="rb")
                    nc.sync.dma_start(rb[:], ors1[q2 * P:(q2 + 1) * P, :])
                    nc.sync.dma_start(out[q2 * P:(q2 + 1) * P, :], rb[:])
        else:
            nc.sync.dma_start(out[tt * P:(tt + 1) * P, :], osb[:])
    if use_rs:
        nc.gpsimd.collective_compute(
            "ReduceScatter", mybir.AluOpType.add,
            replica_groups=RG_PAIRS,
            ins=[obounce[T // 2:T, :].opt()], outs=[ors2.opt()])
        for q2 in range(2):
            rb = op_.tile([P, D], BF16, tag="rb")
            nc.sync.dma_start(rb[:], ors2[q2 * P:(q2 + 1) * P, :])
            nc.sync.dma_start(
                out[T // 4 + q2 * P:T // 4 + (q2 + 1) * P, :], rb[:])
        dp_cm.__exit__(None, None, None)


def _build(maskout: bool, heads_per_core: int = 8, use_rs: bool = True,
           repeat: int = 1, loop_reps: int = 0, phases=None):
    """Build + compile the SPMD program (signature kept for test.py)."""
    del heads_per_core, phases
    nc = bacc.Bacc("TRN2", target_bir_lowering=False, debug=False,
                   num_devices=N_CORES)

    OUT_ROWS = T // 2 if use_rs else T
    qT = nc.dram_tensor("qT", [D, T], FP8, kind="ExternalInput").ap()
    kT = nc.dram_tensor("kT", [D, T], FP8, kind="ExternalInput").ap()
    vT = nc.dram_tensor("vT", [D, T], BF16, kind="ExternalInput").ap()
    wq = nc.dram_tensor("wq", [D, WCOLS], FP8, kind="ExternalInput").ap()
    wk = nc.dram_tensor("wk", [D, WCOLS], FP8, kind="ExternalInput").ap()
    wv = nc.dram_tensor("wv", [D, WCOLS], BF16, kind="ExternalInput").ap()
    wo = nc.dram_tensor("wo", [WCOLS, D], BF16, kind="ExternalInput").ap()
    tri = nc.dram_tensor("tri", [P, WCOLS], FP8, kind="ExternalInput").ap()
    ones = nc.dram_tensor("ones", [P, 1], BF16, kind="ExternalInput").ap()
    ones8d = nc.dram_tensor("ones8d", [P, 1], FP8,
                            kind="ExternalInput").ap()
    out = nc.dram_tensor("out", [OUT_ROWS, D], BF16,
                         kind="ExternalOutput").ap()
    aps = (qT, kT, vT, wq, wk, wv, wo, tri, ones, ones8d, out)

    with tile.TileContext(nc) as tc:
        with (
            tc.tile_pool(name="persist", bufs=1) as pp,
            tc.tile_pool(name="trans", bufs=6) as tp,
            tc.tile_pool(name="osb", bufs=3) as op_,
            tc.tile_pool(name="psum_big", bufs=2, space="PSUM") as psb,
            tc.tile_pool(name="psum_fill", bufs=1, space="PSUM") as psf,
            tc.tile_pool(name="psum_ctx", bufs=1, space="PSUM") as psc,
        ):
            pools = (pp, tp, op_, psb, psf, psc)
            if loop_reps:
                assert not use_rs, "collectives cannot live inside For_i"
                with tc.For_i(0, loop_reps, 1):
                    _emit_body(nc, tc, aps, pools, maskout, use_rs, 0)
            else:
                for rep in range(repeat):
                    _emit_body(nc, tc, aps, pools, maskout, use_rs, rep)

    nc.compile()
    nc.m = get_hw_module(nc.m)
    return nc


_CACHE: dict = {}


def _get_program(maskout: bool):
    key = maskout
    if key not in _CACHE:
        _CACHE[key] = _build(maskout)
    return _CACHE[key]


def _prep_inputs(Q, K, V, Wq, Wk, Wv, Wo, heads_per_core=8):
    """Host-side sharding: layout + bf16 casts, per-core input dicts."""
    del heads_per_core
    keep = np.arange(P)[:, None] >= (np.arange(WCOLS)[None, :] % DK)
    tri = np.where(keep, 0.0, -16.0).astype(F8NP)
    ones = np.ones((P, 1), BFNP)
    ones8 = np.ones((P, 1), F8NP)
    in_maps = []
    for c in range(N_CORES):
        b, g = c // 2, c % 2
        hsel = np.arange(g * HC, (g + 1) * HC)
        wq_p = np.ascontiguousarray(
            Wq[hsel].transpose(1, 0, 2).reshape(D, WCOLS)).astype(F8NP)
        wk_p = np.ascontiguousarray(
            Wk[hsel].transpose(1, 0, 2).reshape(D, WCOLS)).astype(F8NP)
        wv_p = np.ascontiguousarray(
            Wv[hsel].transpose(1, 0, 2).reshape(D, WCOLS)).astype(BFNP)
        wo_p = np.ascontiguousarray(
            Wo.reshape(H, DK, D)[hsel].reshape(WCOLS, D)).astype(BFNP)
        in_maps.append({
            "qT": np.ascontiguousarray(Q[b].T).astype(F8NP),
            "kT": np.ascontiguousarray(K[b].T).astype(F8NP),
            "vT": np.ascontiguousarray(V[b].T).astype(BFNP),
            "wq": wq_p, "wk": wk_p, "wv": wv_p, "wo": wo_p,
            "tri": tri, "ones": ones, "ones8d": ones8,
        })
    return in_maps


def run(Q, K, V, Wq, Wk, Wv, Wo, maskout):
    Q = np.asarray(Q, np.float32)
    K = np.asarray(K, np.float32)
    V = np.asarray(V, np.float32)
    Wq = np.asarray(Wq, np.float32)
    Wk = np.asarray(Wk, np.float32)
    Wv = np.asarray(Wv, np.float32)
    Wo = np.asarray(Wo, np.float32)
    mk = bool(np.asarray(maskout).item())
    nc = _get_program(mk)
    in_maps = _prep_inputs(Q, K, V, Wq, Wk, Wv, Wo)
    res = bass_utils.run_bass_kernel_spmd(
        nc, in_maps, list(range(N_CORES)), trace=False)
    outf = np.empty((B, T, D), np.float32)
    for c in range(N_CORES):
        b, r = c // 2, c % 2
        o = np.asarray(res.results[c]["out"]).astype(np.float32)
        outf[b, r * (T // 4):(r + 1) * (T // 4), :] = o[:T // 4]
        outf[b, T // 2 + r * (T // 4):T // 2 + (r + 1) * (T // 4), :] = \
            o[T // 4:]
    return outf, res


def kernel(Q, K, V, Wq, Wk, Wv, Wo, maskout):
    outf, _ = run(Q, K, V, Wq, Wk, Wv, Wo, maskout)
    return outf


# revision 43
# speedup vs baseline: 1.3933x; 1.2259x over previous
"""Trainium2 Bass kernel for nn_MultiHeadAttention_64647847739885.

Reference semantics (fp32):
    Wq_eff = softmax(Wq + tril_mask, axis=-2)   (if maskout else Wq)  [H,D,DK]
    Wk_eff = softmax(Wk + tril_mask, axis=-2)
    WqQ = einsum('btd,hdk->bhtk', Q, Wq_eff)
    WkK = einsum('bsd,hdk->bhsk', K, Wk_eff)
    WvV = einsum('bsd,hdv->bhsv', V, Wv)
    scores = einsum('bhtk,bhsk->bhts', WqQ, WkK) / sqrt(dk)
    probs = softmax(scores, axis=-2)            # over the QUERY axis t!
    ctx = einsum('bhts,bhsv->bhtv', probs, WvV) -> (B,T,H*DV) @ Wo

Device strategy (8 NeuronCores, SPMD): core c owns batch b = c//2 and
head-group g = c%2 (8 heads each).  Each core computes attention + the
partial output projection (its 8 heads, all T rows) and the pair combines
partials with two pairwise bf16 ReduceScatters, each core emitting its
T/2 rows of the output.

Pipeline layout:
  - host casts: V/Wv/Wo travel bf16; Q/K and the pre-softmax Wq/Wk travel
    fp8e4 (safe: their rounding flows only through the tiny-score exp
    channel; V-side stays bf16 to protect the direct channel);
  - q/k projections contract via fp8 DoubleRow (two 128-deep k-subtiles
    per matmul); attention scores/ctx and everything else stay bf16;
  - DMA queues: SP carries wq/vT/qT/kT + output, Pool carries
    wk/wv/wo/consts; the ACT engine runs ONLY the exp chain;
  - attention emits both heads' scores matmuls adjacently on disjoint
    PE row-groups and both ctx matmuls on disjoint col-groups
    (tile_position), so the halves overlap in the array; ctx is deferred
    one s-tile and the next pair's projections interleave as PE fillers
    so the PE never waits on the exp;
  - psum: 2-deep scores ring (2x2 banks) + 1 filler group + ctx
    accumulator = 8 banks; mask uses -16 (exp(-16)~0, fp8-safe).
All softmax denominators fold into per-partition scales as in the
reference factorization.  Host does layout + dtype-cast work only.
"""

import numpy as np
import ml_dtypes

import concourse.bacc as bacc
import concourse.mybir as mybir
import concourse.tile as tile
from concourse import bass_utils
from concourse.bass_interp import get_hw_module

B, T, D = 4, 1024, 1024
H, DK = 16, 64
P = 128
N_CORES = 8
HC = 8               # heads per core
NPAIR = HC // 2      # head-pairs per core (ctx partition groups)
WCOLS = HC * DK      # packed weight columns per core (512)
ND = D // P          # contraction tiles for projections
NS = T // P          # s tiles
NT2 = T // 512       # moving-dim halves

F32 = mybir.dt.float32
BF16 = mybir.dt.bfloat16
FP8 = mybir.dt.float8e4
BFNP = ml_dtypes.bfloat16
F8NP = ml_dtypes.float8_e4m3

RG_PAIRS = [[0, 1], [2, 3], [4, 5], [6, 7]]

EXP = mybir.ActivationFunctionType.Exp


def _emit_body(nc, tc, aps, pools, maskout, use_rs, rep):
    qT, kT, vT, wq, wk, wv, wo, tri, ones, ones8d, out = aps
    pp, tp, op_, psb, psf, psc = pools

    ones_t = pp.tile([P, 1], BF16, tag="ones")
    ones8 = pp.tile([P, 1], FP8, tag="ones8")
    qT_t = pp.tile([P, ND, T], FP8, tag="qT")
    kT_t = pp.tile([P, ND, T], FP8, tag="kT")
    vT_t = pp.tile([P, ND, T], BF16, tag="vT")
    wq_t = pp.tile([P, ND, WCOLS], FP8, tag="wq")
    wk_t = pp.tile([P, ND, WCOLS], FP8, tag="wk")
    wv_t = pp.tile([P, ND, WCOLS], BF16, tag="wv")
    wvv = pp.tile([P, NS, WCOLS], BF16, tag="wvv")
    qq = pp.tile([P, NPAIR, T], BF16, tag="qq")
    kk = pp.tile([P, NPAIR, T], BF16, tag="kk")
    ctx = pp.tile([P, NPAIR, T], BF16, tag="ctx")
    wo_t = pp.tile([P, NPAIR, D], BF16, tag="wo")
    wst_q = pp.tile([P, ND, WCOLS], FP8, tag="wstq")
    wst_k = pp.tile([P, ND, WCOLS], FP8, tag="wstk")
    if maskout:
        tri_t = pp.tile([P, WCOLS], FP8, tag="tri")

    # ---------------- DMA enqueues (SP / Pool queues) ---------------
    # SP: wq chunks -> vT -> qT -> kT (+ output later); Pool: consts,
    # wk, wv, wo.  ACT issues no DMA at all.
    wq_dst = wst_q
    wk_dst = wst_k
    if maskout:
        nc.gpsimd.dma_start(tri_t[:], tri[:])
    nc.gpsimd.dma_start(ones_t[:], ones[:])
    nc.gpsimd.dma_start(ones8[:], ones8d[:])
    for i in range(ND):
        nc.sync.dma_start(wq_dst[:, i, :], wq[i * P:(i + 1) * P, :])
    for i in range(ND):
        nc.sync.dma_start(vT_t[:, i, :], vT[i * P:(i + 1) * P, :])
    for i in range(ND):
        nc.gpsimd.dma_start(wk_dst[:, i, :], wk[i * P:(i + 1) * P, :])
    for i in range(ND):
        nc.gpsimd.dma_start(wv_t[:, i, :], wv[i * P:(i + 1) * P, :])
    for i in range(ND):
        nc.sync.dma_start(qT_t[:, i, :], qT[i * P:(i + 1) * P, :])
    for i in range(ND):
        nc.sync.dma_start(kT_t[:, i, :], kT[i * P:(i + 1) * P, :])
    for m in range(NPAIR):
        nc.gpsimd.dma_start(wo_t[:, m, :], wo[m * P:(m + 1) * P, :])

    # ---------------- weight softmax -------------------------------
    # additive mask (tri holds 0 / -1e4) then exp (ACT); the softmax
    # denominators become per-partition scales on qq via ones-matmul
    # column sums + PE transposes.
    cscale = [None] * NPAIR
    if maskout:
        nc.vector.tensor_add(wst_q[:, 0, :], wst_q[:, 0, :], tri_t[:])
        nc.vector.tensor_add(wst_k[:, 0, :], wst_k[:, 0, :], tri_t[:])
        for i in range(ND):
            nc.scalar.activation(wq_t[:, i, :], wst_q[:, i, :], EXP)
        for i in range(ND):
            nc.scalar.activation(wk_t[:, i, :], wst_k[:, i, :], EXP)
    else:
        for i in range(ND):
            nc.vector.tensor_copy(wq_t[:, i, :], wst_q[:, i, :])
        for i in range(ND):
            nc.vector.tensor_copy(wk_t[:, i, :], wst_k[:, i, :])

    # ---------------- softmax denominators -> cscale ----------------
    if maskout:
        ps_s = psf.tile([P, 1024], F32, tag="f")
        for i in range(ND):
            nc.tensor.matmul(ps_s[:1, 0:WCOLS], lhsT=ones8[:],
                             rhs=wq_t[:, i, :],
                             start=(i == 0), stop=(i == ND - 1))
        for i in range(ND):
            nc.tensor.matmul(ps_s[:1, WCOLS:T], lhsT=ones8[:],
                             rhs=wk_t[:, i, :],
                             start=(i == 0), stop=(i == ND - 1))
        ssb = tp.tile([1, T], BF16, tag="ssb")
        nc.vector.tensor_copy(ssb[:], ps_s[:1, :])
        ps_t = psf.tile([P, 1024], F32, tag="f")
        for pr in range(NPAIR):
            nc.tensor.matmul(ps_t[:, pr:pr + 1],
                             lhsT=ssb[:, pr * P:(pr + 1) * P],
                             rhs=ones_t[:1, :], start=True, stop=True)
            nc.tensor.matmul(
                ps_t[:, 4 + pr:5 + pr],
                lhsT=ssb[:, WCOLS + pr * P:WCOLS + (pr + 1) * P],
                rhs=ones_t[:1, :], start=True, stop=True)
        sqk = tp.tile([P, 2 * NPAIR], F32, tag="sqk")
        nc.vector.tensor_copy(sqk[:], ps_t[:, 0:2 * NPAIR])
        prod = tp.tile([P, NPAIR], F32, tag="prod")
        nc.vector.tensor_mul(prod[:], sqk[:, 0:NPAIR], sqk[:, NPAIR:])
        call = pp.tile([P, NPAIR], F32, tag="call")
        nc.vector.reciprocal(call[:], prod[:])
        for pr in range(NPAIR):
            cscale[pr] = call[:, pr:pr + 1]

    # ---------------- wvv = (V @ Wv) in (s x v), bf16 ---------------
    for st in range(NS):
        ps = psb.tile([P, 1024], F32, tag="big")
        for i in range(ND):
            nc.tensor.matmul(ps[:, :WCOLS],
                             lhsT=vT_t[:, i, st * P:(st + 1) * P],
                             rhs=wv_t[:, i, :],
                             start=(i == 0), stop=(i == ND - 1))
        nc.vector.tensor_copy(wvv[:, st, :], ps[:, :WCOLS])

    # ---------------- q/k projection emitters -----------------------
    # Returned as a flat list of thunks (16 matmuls + evac) so the
    # attention loop can interleave them as PE fillers.
    def proj_thunks(pr, which, pool=None):
        w_t = wq_t if which == 'q' else wk_t
        x_t = qT_t if which == 'q' else kT_t
        dst = qq if which == 'q' else kk
        pool_, tag = (pool or psf), ("big" if pool is psb else "f")
        state = {}

        def mk(k):
            def f():
                if k == 0:
                    state['ps'] = pool_.tile([P, 1024], F32, tag=tag,
                                             name="ps_fill")
                j, n = divmod(k, NT2)
                # fp8 DoubleRow: two 128-deep k-subtiles per matmul
                nc.tensor.matmul(
                    state['ps'][:, n * 512:(n + 1) * 512],
                    lhsT=w_t[:, 2 * j:2 * j + 2, pr * P:(pr + 1) * P],
                    rhs=x_t[:, 2 * j:2 * j + 2, n * 512:(n + 1) * 512],
                    start=(j == 0), stop=(j == ND // 2 - 1),
                    perf_mode=mybir.MatmulPerfMode.DoubleRow)
            return f

        def evac():
            if which == 'q' and cscale[pr] is not None:
                nc.vector.tensor_scalar_mul(dst[:, pr, :], state['ps'][:],
                                            cscale[pr][:])
            else:
                nc.vector.tensor_copy(dst[:, pr, :], state['ps'][:])

        return [mk(k) for k in range(ND // 2 * NT2)] + [evac]

    def run_all(thunks):
        for t_ in thunks:
            t_()

    # ---------------- attention (software-pipelined) ----------------
    def attn_pair(pr, fillers):
        fi = 0
        pctx = psc.tile([P, T], F32, tag="ctxp")
        pend = None

        def emit_ctx2(e0, wvs0, e1, wvs1, st):
            # both halves adjacent: disjoint array col-groups overlap on HW
            for n in range(NT2):
                nc.tensor.matmul(
                    pctx[0:64, n * 512:(n + 1) * 512],
                    lhsT=wvs0[:], rhs=e0[:, n * 512:(n + 1) * 512],
                    start=(st == 0), stop=(st == NS - 1),
                    tile_position=(0, 0))
                nc.tensor.matmul(
                    pctx[64:128, n * 512:(n + 1) * 512],
                    lhsT=wvs1[:], rhs=e1[:, n * 512:(n + 1) * 512],
                    start=(st == 0), stop=(st == NS - 1),
                    tile_position=(0, 64))

        for st in range(NS):
            # scores for both heads adjacent: disjoint array row-groups
            ps0 = psb.tile([P, 1024], F32, tag="big", name="ps0")
            ps1 = psb.tile([P, 1024], F32, tag="big", name="ps1")
            for n in range(NT2):
                nc.tensor.matmul(
                    ps0[:, n * 512:(n + 1) * 512],
                    lhsT=kk[0:64, pr, st * P:(st + 1) * P],
                    rhs=qq[0:64, pr, n * 512:(n + 1) * 512],
                    start=True, stop=True, tile_position=(0, 0))
                nc.tensor.matmul(
                    ps1[:, n * 512:(n + 1) * 512],
                    lhsT=kk[64:128, pr, st * P:(st + 1) * P],
                    rhs=qq[64:128, pr, n * 512:(n + 1) * 512],
                    start=True, stop=True, tile_position=(64, 0))
            e0 = tp.tile([P, T], BF16, tag="e", name="e0")
            rs0 = tp.tile([P, 1], F32, tag="rs", name="rs0")
            nc.scalar.activation(e0[:], ps0[:], EXP, scale=0.125,
                                 accum_out=rs0[:])
            r0 = tp.tile([P, 1], F32, tag="r", name="r0")
            nc.vector.reciprocal(r0[:], rs0[:])
            wvs0 = tp.tile([P, DK], BF16, tag="wvs", name="wvs0")
            hcol = 2 * pr * DK
            nc.vector.tensor_scalar_mul(wvs0[:],
                                        wvv[:, st, hcol:hcol + DK], r0[:])
            e1 = tp.tile([P, T], BF16, tag="e", name="e1")
            rs1 = tp.tile([P, 1], F32, tag="rs", name="rs1")
            nc.scalar.activation(e1[:], ps1[:], EXP, scale=0.125,
                                 accum_out=rs1[:])
            r1 = tp.tile([P, 1], F32, tag="r", name="r1")
            nc.vector.reciprocal(r1[:], rs1[:])
            wvs1 = tp.tile([P, DK], BF16, tag="wvs", name="wvs1")
            hcol1 = (2 * pr + 1) * DK
            nc.vector.tensor_scalar_mul(wvs1[:],
                                        wvv[:, st, hcol1:hcol1 + DK], r1[:])
            nfill = 5 if fi < 30 else 4
            for _ in range(nfill):
                if fi < len(fillers):
                    fillers[fi]()
                    fi += 1
            if pend is not None:
                emit_ctx2(*pend)
            pend = (e0, wvs0, e1, wvs1, st)
        emit_ctx2(*pend)
        while fi < len(fillers):
            fillers[fi]()
            fi += 1
        nc.vector.tensor_copy(ctx[:, pr, :], pctx[:])

    run_all(proj_thunks(0, 'q'))
    run_all(proj_thunks(0, 'k', pool=psb))
    attn_pair(0, proj_thunks(1, 'q') + proj_thunks(1, 'k'))
    attn_pair(1, proj_thunks(2, 'q') + proj_thunks(2, 'k'))
    attn_pair(2, proj_thunks(3, 'q') + proj_thunks(3, 'k'))
    attn_pair(3, [])

    # ---------------- output projection (bf16 partial, all T rows) --
    # pairwise bf16 ReduceScatter combines head-group partials; rank r
    # of each pair receives rows [r*256,(r+1)*256) of each T/2 half.
    if use_rs:
        dp_cm = tc.tile_pool(name=f"dram{rep}", bufs=1, space="DRAM")
        dp = dp_cm.__enter__()
        obounce = dp.tile([T, D], BF16, tag="ob")
        ors1 = dp.tile([T // 4, D], BF16, tag="ors1")
        ors2 = dp.tile([T // 4, D], BF16, tag="ors2")
    for tt in range(T // P):
        pso = psb.tile([P, 1024], F32, tag="big")
        for m in range(NPAIR):
            for n in range(NT2):
                nc.tensor.matmul(
                    pso[:, n * 512:(n + 1) * 512],
                    lhsT=ctx[:, m, tt * P:(tt + 1) * P],
                    rhs=wo_t[:, m, n * 512:(n + 1) * 512],
                    start=(m == 0), stop=(m == NPAIR - 1))
        osb = op_.tile([P, D], BF16, tag="o")
        nc.vector.tensor_copy(osb[:], pso[:])
        if use_rs:
            nc.sync.dma_start(obounce[tt * P:(tt + 1) * P, :], osb[:])
            if tt == T // P // 2 - 1:
                # first-half RS overlaps the second half's projection
                nc.gpsimd.collective_compute(
                    "ReduceScatter", mybir.AluOpType.add,
                    replica_groups=RG_PAIRS,
                    ins=[obounce[0:T // 2, :].opt()], outs=[ors1.opt()])
                for q2 in range(2):
                    rb = op_.tile([P, D], BF16, tag="rb")
                    nc.sync.dma_start(rb[:], ors1[q2 * P:(q2 + 1) * P, :])
                    nc.sync.dma_start(out[q2 * P:(q2 + 1) * P, :], rb[:])
        else:
            nc.sync.dma_start(out[tt * P:(tt + 1) * P, :], osb[:])
    if use_rs:
        nc.gpsimd.collective_compute(
            "ReduceScatter", mybir.AluOpType.add,
            replica_groups=RG_PAIRS,
            ins=[obounce[T // 2:T, :].opt()], outs=[ors2.opt()])
        for q2 in range(2):
            rb = op_.tile([P, D], BF16, tag="rb")
            nc.sync.dma_start(rb[:], ors2[q2 * P:(q2 + 1) * P, :])
            nc.sync.dma_start(
                out[T // 4 + q2 * P:T // 4 + (q2 + 1) * P, :], rb[:])
        dp_cm.__exit__(None, None, None)


def _build(maskout: bool, heads_per_core: int = 8, use_rs: bool = True,
           repeat: int = 1, loop_reps: int = 0, phases=None):
    """Build + compile the SPMD program (signature kept for test.py)."""
    del heads_per_core, phases
    nc = bacc.Bacc("TRN2", target_bir_lowering=False, debug=False,
                   num_devices=N_CORES)

    OUT_ROWS = T // 2 if use_rs else T
    qT = nc.dram_tensor("qT", [D, T], FP8, kind="ExternalInput").ap()
    kT = nc.dram_tensor("kT", [D, T], FP8, kind="ExternalInput").ap()
    vT = nc.dram_tensor("vT", [D, T], BF16, kind="ExternalInput").ap()
    wq = nc.dram_tensor("wq", [D, WCOLS], FP8, kind="ExternalInput").ap()
    wk = nc.dram_tensor("wk", [D, WCOLS], FP8, kind="ExternalInput").ap()
    wv = nc.dram_tensor("wv", [D, WCOLS], BF16, kind="ExternalInput").ap()
    wo = nc.dram_tensor("wo", [WCOLS, D], BF16, kind="ExternalInput").ap()
    tri = nc.dram_tensor("tri", [P, WCOLS], FP8, kind="ExternalInput").ap()
    ones = nc.dram_tensor("ones", [P, 1], BF16, kind="ExternalInput").ap()
    ones8d = nc.dram_tensor("ones8d", [P, 1], FP8,
                            kind="ExternalInput").ap()
    out = nc.dram_tensor("out", [OUT_ROWS, D], BF16,
                         kind="ExternalOutput").ap()
    aps = (qT, kT, vT, wq, wk, wv, wo, tri, ones, ones8d, out)

    with tile.TileContext(nc) as tc:
        with (
            tc.tile_pool(name="persist", bufs=1) as pp,
            tc.tile_pool(name="trans", bufs=6) as tp,
            tc.tile_pool(name="osb", bufs=3) as op_,
            tc.tile_pool(name="psum_big", bufs=2, space="PSUM") as psb,
            tc.tile_pool(name="psum_fill", bufs=1, space="PSUM") as psf,
            tc.tile_pool(name="psum_ctx", bufs=1, space="PSUM") as psc,
        ):
            pools = (pp, tp, op_, psb, psf, psc)
            if loop_reps:
                assert not use_rs, "collectives cannot live inside For_i"
                with tc.For_i(0, loop_reps, 1):
                    _emit_body(nc, tc, aps, pools, maskout, use_rs, 0)
            else:
                for rep in range(repeat):
                    _emit_body(nc, tc, aps, pools, maskout, use_rs, rep)

    nc.compile()
    nc.m = get_hw_module(nc.m)
    return nc


_CACHE: dict = {}


def _get_program(maskout: bool):
    key = maskout
    if key not in _CACHE:
        _CACHE[key] = _build(maskout)
    return _CACHE[key]


def _prep_inputs(Q, K, V, Wq, Wk, Wv, Wo, heads_per_core=8):
    """Host-side sharding: layout + bf16 casts, per-core input dicts."""
    del heads_per_core
    keep = np.arange(P)[:, None] >= (np.arange(WCOLS)[None, :] % DK)
    tri = np.where(keep, 0.0, -16.0).astype(F8NP)
    ones = np.ones((P, 1), BFNP)
    ones8 = np.ones((P, 1), F8NP)
    in_maps = []
    for c in range(N_CORES):
        b, g = c // 2, c % 2
        hsel = np.arange(g * HC, (g + 1) * HC)
        wq_p = np.ascontiguousarray(
            Wq[hsel].transpose(1, 0, 2).reshape(D, WCOLS)).astype(F8NP)
        wk_p = np.ascontiguousarray(
            Wk[hsel].transpose(1, 0, 2).reshape(D, WCOLS)).astype(F8NP)
        wv_p = np.ascontiguousarray(
            Wv[hsel].transpose(1, 0, 2).reshape(D, WCOLS)).astype(BFNP)
        wo_p = np.ascontiguousarray(
            Wo.reshape(H, DK, D)[hsel].reshape(WCOLS, D)).astype(BFNP)
        in_maps.append({
            "qT": np.ascontiguousarray(Q[b].T).astype(F8NP),
            "kT": np.ascontiguousarray(K[b].T).astype(F8NP),
            "vT": np.ascontiguousarray(V[b].T).astype(BFNP),
            "wq": wq_p, "wk": wk_p, "wv": wv_p, "wo": wo_p,
            "tri": tri, "ones": ones, "ones8d": ones8,
        })
    return in_maps


def run(Q, K, V, Wq, Wk, Wv, Wo, maskout):
    Q = np.asarray(Q, np.float32)
    K = np.asarray(K, np.float32)
    V = np.asarray(V, np.float32)
    Wq = np.asarray(Wq, np.float32)
    Wk = np.asarray(Wk, np.float32)
    Wv = np.asarray(Wv, np.float32)
    Wo = np.asarray(Wo, np.float32)
    mk = bool(np.asarray(maskout).item())
    nc = _get_program(mk)
    in_maps = _prep_inputs(Q, K, V, Wq, Wk, Wv, Wo)
    res = bass_utils.run_bass_kernel_spmd(
        nc, in_maps, list(range(N_CORES)), trace=False)
    outf = np.empty((B, T, D), np.float32)
    for c in range(N_CORES):
        b, r = c // 2, c % 2
        o = np.asarray(res.results[c]["out"]).astype(np.float32)
        outf[b, r * (T // 4):(r + 1) * (T // 4), :] = o[:T // 4]
        outf[b, T // 2 + r * (T // 4):T // 2 + (r + 1) * (T // 4), :] = \
            o[T // 4:]
    return outf, res


def kernel(Q, K, V, Wq, Wk, Wv, Wo, maskout):
    outf, _ = run(Q, K, V, Wq, Wk, Wv, Wo, maskout)
    return outf
